# revision 1
# baseline (speedup 1.0000x reference)
"""Trainium2 Bass kernel for a soft-MoE (MANN) block.

Reference math (per token b):
    g  = elu(x_gate @ g1_w.T + g1_b); g = elu(g @ g2_w.T + g2_b)
    ew = softmax(g @ g3_w.T + g3_b)                      # [B, K=8]
    h1 = elu(sum_k ew_k * (x_main @ W1_k.T) + ew @ b1)   # [B, 1024]
    h2 = elu(sum_k ew_k * (h1 @ W2_k.T) + ew @ b2)       # [B, 1024]
    y  =     sum_k ew_k * (h2 @ W3_k.T) + ew @ b3        # [B, 640]

Strategy: data-parallel over 8 NeuronCores (128 batch rows per core),
expert weights replicated, streamed from HBM in bf16 (fp32 accumulate).
The per-expert combine is folded into PSUM accumulation by scaling the
layer *input* with ew_k before the matmul ("scale-before"), so each
output chunk is one PSUM accumulation group over (expert, i-tile).
Gating runs entirely in fp32.
"""

import sys

sys.path.insert(0, "/opt/trn_rl_repo")

from contextlib import ExitStack

import numpy as np
import ml_dtypes

import concourse.bass as bass
from concourse import bacc
import concourse.tile as tile
from concourse import mybir
from concourse.bass_utils import run_bass_kernel_spmd
from concourse.masks import make_identity

F32 = mybir.dt.float32
BF16 = mybir.dt.bfloat16
AF = mybir.ActivationFunctionType
OP = mybir.AluOpType

B = 1024
X_MAIN, X_GATE, Y_DIM = 480, 128, 640
HID, GHID, K = 1024, 64, 8
NCORES = 8
BS = B // NCORES  # 128 batch rows per core

# trunk layer configs: (partition size of i-tiles, #i-tiles, O, o-chunk sizes)
L1 = (120, 4, HID, (512, 512))
L2 = (128, 8, HID, (512, 512))
L3 = (128, 8, Y_DIM, (512, 128))


def _build_program(with_bias: tuple[bool, bool, bool]) -> bass.Bass:
    nc = bacc.Bacc()

    # ---- DRAM parameters (host supplies exactly these layouts) ----
    xm_ext = nc.declare_dram_parameter("xm", [120, 4, BS], F32, isOutput=False)
    xg_ext = nc.declare_dram_parameter("xg", [X_GATE, BS], F32, isOutput=False)
    g1w_ext = nc.declare_dram_parameter("g1w", [X_GATE, GHID], F32, isOutput=False)
    g1b_ext = nc.declare_dram_parameter("g1b", [GHID, 1], F32, isOutput=False)
    g2w_ext = nc.declare_dram_parameter("g2w", [GHID, GHID], F32, isOutput=False)
    g2b_ext = nc.declare_dram_parameter("g2b", [GHID, 1], F32, isOutput=False)
    g3w_ext = nc.declare_dram_parameter("g3w", [GHID, K], F32, isOutput=False)
    g3b_ext = nc.declare_dram_parameter("g3b", [1, K], F32, isOutput=False)
    w_ext = []
    b_ext = []
    for li, (P, IT, O, _) in enumerate((L1, L2, L3)):
        w_ext.append(
            nc.declare_dram_parameter(f"w{li + 1}", [K, P, IT, O], BF16, isOutput=False)
        )
        if with_bias[li]:
            b_ext.append(
                nc.declare_dram_parameter(f"b{li + 1}", [K, O], F32, isOutput=False)
            )
        else:
            b_ext.append(None)
    y_ext = nc.declare_dram_parameter("y", [BS, Y_DIM], F32, isOutput=True)

    with TileCtx(nc) as tc, ExitStack() as ctx:
        const = ctx.enter_context(tc.tile_pool(name="const", bufs=1))
        gat = ctx.enter_context(tc.tile_pool(name="gat", bufs=1))
        spsum = ctx.enter_context(tc.tile_pool(name="spsum", bufs=2, space="PSUM"))
        zpsum = ctx.enter_context(tc.tile_pool(name="zpsum", bufs=3, space="PSUM"))
        tpsum = ctx.enter_context(tc.tile_pool(name="tpsum", bufs=2, space="PSUM"))
        xpool = ctx.enter_context(tc.tile_pool(name="xpool", bufs=1))
        xkp = ctx.enter_context(tc.tile_pool(name="xkp", bufs=2))
        hscr = ctx.enter_context(tc.tile_pool(name="hscr", bufs=1))
        hpool = ctx.enter_context(tc.tile_pool(name="hpool", bufs=2))
        wp = [
            ctx.enter_context(tc.tile_pool(name="w1p", bufs=5)),
            ctx.enter_context(tc.tile_pool(name="w2p", bufs=4)),
            ctx.enter_context(tc.tile_pool(name="w3p", bufs=2)),
        ]

        ident = const.tile([128, 128], F32)
        make_identity(nc, ident)
        ones = const.tile([1, BS], F32)
        nc.vector.memset(ones, 1.0)

        # ---------------- gating (fp32) ----------------
        xg_sb = gat.tile([X_GATE, BS], F32)
        nc.sync.dma_start(xg_sb, xg_ext[:])
        g1w_sb = gat.tile([X_GATE, GHID], F32)
        nc.sync.dma_start(g1w_sb, g1w_ext[:])
        g1b_sb = gat.tile([GHID, 1], F32)
        nc.sync.dma_start(g1b_sb, g1b_ext[:])
        g2w_sb = gat.tile([GHID, GHID], F32)
        nc.sync.dma_start(g2w_sb, g2w_ext[:])
        g2b_sb = gat.tile([GHID, 1], F32)
        nc.sync.dma_start(g2b_sb, g2b_ext[:])
        g3w_sb = gat.tile([GHID, K], F32)
        nc.sync.dma_start(g3w_sb, g3w_ext[:])
        g3b_sb = gat.tile([1, K], F32)
        nc.sync.dma_start(g3b_sb, g3b_ext[:])

        def gate_elup(zp, bias_sb, name):
            # returns elu(z + bias) + 1 = relu(z+bias) + exp(min(z+bias, 0)), [GHID, BS] f32
            r = gat.tile([GHID, BS], F32, tag=f"r_{name}")
            nc.scalar.activation(r, zp, AF.Relu, bias=g_ap(bias_sb))
            m = gat.tile([GHID, BS], F32, tag=f"m_{name}")
            nc.vector.tensor_scalar(m, zp, g_ap(bias_sb), 0.0, OP.add, OP.min)
            e = gat.tile([GHID, BS], F32, tag=f"e_{name}")
            nc.scalar.activation(e, m, AF.Exp)
            hp = gat.tile([GHID, BS], F32, tag=f"hp_{name}")
            nc.vector.tensor_tensor(hp, r, e, OP.add)
            return hp

        def g_ap(t):
            return t[:, 0:1]

        zg1 = spsum.tile([GHID, BS], F32, tag="g")
        nc.tensor.matmul(zg1, lhsT=g1w_sb, rhs=xg_sb, start=True, stop=True)
        h1p = gate_elup(zg1, g1b_sb, "g1")

        zg2 = spsum.tile([GHID, BS], F32, tag="g")
        nc.tensor.matmul(zg2, lhsT=g2w_sb, rhs=h1p, start=True, stop=True)
        h2p = gate_elup(zg2, g2b_sb, "g2")

        # logits in [b, k] layout: lhsT = h2p [GHID, BS], rhs = g3w [GHID, K]
        zg3 = spsum.tile([BS, K], F32, tag="g")
        nc.tensor.matmul(zg3, lhsT=h2p, rhs=g3w_sb, start=True, stop=False)
        nc.tensor.matmul(zg3, lhsT=ones, rhs=g3b_sb, start=False, stop=True)

        # softmax along free dim (K)
        negmx = gat.tile([BS, 1], F32)
        nc.vector.tensor_reduce(negmx, zg3, mybir.AxisListType.X, OP.max, negate=True)
        e3 = gat.tile([BS, K], F32)
        ssum = gat.tile([BS, 1], F32)
        nc.scalar.activation(e3, zg3, AF.Exp, bias=negmx[:, 0:1], accum_out=ssum[:, 0:1])
        rcp = gat.tile([BS, 1], F32)
        nc.vector.reciprocal(rcp, ssum)
        ewT = gat.tile([BS, K], F32)  # [b, k]
        nc.vector.tensor_scalar_mul(ewT, e3, rcp[:, 0:1])

        # per-expert row at partition 0: ew_rows[0, k, :] = ewT[:, k].T
        ew_rows = gat.tile([1, K, BS], F32)
        for k in range(K):
            rp = spsum.tile([1, BS], F32, tag="g")
            nc.tensor.transpose(rp, ewT[:, k : k + 1], ident)
            nc.vector.tensor_copy(out=ew_rows[:, k, :], in_=rp)

        # broadcast rows: ewb[:, k, :] = ew_k replicated over all 128 partitions
        ewb = gat.tile([128, K, BS], F32)
        for k in range(K):
            bp = spsum.tile([128, BS], F32, tag="g")
            nc.tensor.matmul(
                bp, lhsT=ones, rhs=ew_rows[:, k, :], start=True, stop=True
            )
            nc.vector.tensor_copy(out=ewb[:, k, :], in_=bp)

        if any(with_bias):
            # ew [K, BS] on partitions 0..K-1 (lhsT for the bias matmuls)
            ewps = spsum.tile([K, BS], F32, tag="g")
            nc.tensor.transpose(ewps, ewT, ident)
            ew_sb = gat.tile([K, BS], F32)
            nc.vector.tensor_copy(out=ew_sb, in_=ewps)

        # ---------------- trunk ----------------
        x1_sb = xpool.tile([120, 4, BS], F32, tag="x1")
        nc.sync.dma_start(x1_sb, xm_ext[:])

        x_sb = x1_sb
        for li, (P, IT, O, chunks) in enumerate((L1, L2, L3)):
            last = li == 2
            # scale-before: xk[:, k, it, :] = x * ew_k  (bf16)
            xk = xkp.tile([P, K, IT, BS], BF16, tag="xk", name=f"xk{li}")
            for k in range(K):
                nc.vector.tensor_tensor(
                    xk[:, k],
                    x_sb,
                    ewb[:P, k, None, :].to_broadcast((P, IT, BS)),
                    OP.mult,
                )
            if not last:
                nx_sb = xpool.tile([128, O // 128, BS], F32, tag=f"x{li + 2}")
            if b_ext[li] is not None:
                bl_sb = gat.tile([K, O], F32, tag=f"bias{li}")
                nc.sync.dma_start(bl_sb, b_ext[li][:])

            zps = []
            oc0 = 0
            for ci, ocsz in enumerate(chunks):
                zp = zpsum.tile([BS, 512], F32, tag="z", name=f"zp{li}_{ci}")[:, :ocsz]
                if b_ext[li] is not None:
                    nc.tensor.matmul(
                        zp, lhsT=ew_sb, rhs=bl_sb[:, oc0 : oc0 + ocsz],
                        start=True, stop=False,
                    )
                zps.append((zp, oc0, ocsz))
                oc0 += ocsz
            for k in range(K):
                w_sb = wp[li].tile([P, IT, O], BF16, tag=f"w{li}", name=f"w{li}_{k}")
                nc.sync.dma_start(w_sb, w_ext[li][k])
                for zp, occ, ocsz in zps:
                    for it in range(IT):
                        nc.tensor.matmul(
                            zp,
                            lhsT=xk[:, k, it, :],
                            rhs=w_sb[:, it, occ : occ + ocsz],
                            start=(k == 0 and it == 0 and b_ext[li] is None),
                            stop=(k == K - 1 and it == IT - 1),
                        )
            for zp, oc0, ocsz in zps:
                if last:
                    y_sb = hpool.tile([BS, 512], F32, tag="y", name="y_sb")[:, :ocsz]
                    nc.vector.tensor_copy(out=y_sb, in_=zp)
                    nc.sync.dma_start(y_ext[:, oc0 : oc0 + ocsz], y_sb)
                else:
                    # h = (max(z,0) - 1) + exp(min(z,0))   (= elu(z))
                    m = hscr.tile([BS, 512], F32, tag="hm", name="hm")[:, :ocsz]
                    nc.vector.tensor_scalar(m, zp, 0.0, None, OP.min)
                    e = hscr.tile([BS, 512], F32, tag="he", name="he")[:, :ocsz]
                    nc.scalar.activation(e, m, AF.Exp)
                    r = hscr.tile([BS, 512], F32, tag="hr", name="hr")[:, :ocsz]
                    nc.vector.tensor_scalar(r, zp, 0.0, -1.0, OP.max, OP.add)
                    h = hpool.tile([BS, 512], F32, tag="hh", name="hh")[:, :ocsz]
                    nc.vector.tensor_tensor(h, r, e, OP.add)
                    # transpose each 128-col block into next layer's input layout
                    for j in range(ocsz // 128):
                        tp = tpsum.tile([128, BS], F32, tag="tr")
                        nc.tensor.transpose(tp, h[:, j * 128 : (j + 1) * 128], ident)
                        nc.vector.tensor_copy(
                            out=nx_sb[:, (oc0 // 128) + j, :], in_=tp
                        )
            if not last:
                x_sb = nx_sb

    nc.compile()
    return nc


def TileCtx(nc):
    return tile.TileContext(nc)


_PROG_CACHE: dict = {}


def _get_program(with_bias):
    key = tuple(with_bias)
    if key not in _PROG_CACHE:
        _PROG_CACHE[key] = _build_program(key)
    return _PROG_CACHE[key]


def _prep_w(W, P, IT):
    # [K, O, I] -> [K, P, IT, O] with element [k,p,it,o] = W[k,o,it*P+p]
    Kk, O, I = W.shape
    Wt = W.transpose(0, 2, 1).reshape(Kk, IT, P, O).transpose(0, 2, 1, 3)
    return np.ascontiguousarray(Wt.astype(ml_dtypes.bfloat16))


def kernel(
    x_main, x_gate, g1_w, g1_b, g2_w, g2_b, g3_w, g3_b,
    W1, b1, W2, b2, W3, b3,
):
    x_main = np.asarray(x_main, np.float32)
    x_gate = np.asarray(x_gate, np.float32)
    g1_w = np.asarray(g1_w, np.float32)
    g1_b = np.asarray(g1_b, np.float32)
    g2_w = np.asarray(g2_w, np.float32)
    g2_b = np.asarray(g2_b, np.float32)
    g3_w = np.asarray(g3_w, np.float32)
    g3_b = np.asarray(g3_b, np.float32)
    W1 = np.asarray(W1, np.float32)
    b1 = np.asarray(b1, np.float32)
    W2 = np.asarray(W2, np.float32)
    b2 = np.asarray(b2, np.float32)
    W3 = np.asarray(W3, np.float32)
    b3 = np.asarray(b3, np.float32)

    with_bias = (bool(b1.any()), bool(b2.any()), bool(b3.any()))
    nc = _get_program(with_bias)

    shared = {
        "g1w": np.ascontiguousarray(g1_w.T),
        "g1b": np.ascontiguousarray(g1_b.reshape(GHID, 1)),
        "g2w": np.ascontiguousarray(g2_w.T),
        "g2b": np.ascontiguousarray((g2_b - g2_w.sum(1)).reshape(GHID, 1)),
        "g3w": np.ascontiguousarray(g3_w.T),
        "g3b": np.ascontiguousarray((g3_b - g3_w.sum(1)).reshape(1, K)),
        "w1": _prep_w(W1, 120, 4),
        "w2": _prep_w(W2, 128, 8),
        "w3": _prep_w(W3, 128, 8),
    }
    for name, b, flag in (("b1", b1, with_bias[0]), ("b2", b2, with_bias[1]),
                          ("b3", b3, with_bias[2])):
        if flag:
            shared[name] = np.ascontiguousarray(b)

    in_maps = []
    for s in range(NCORES):
        xm_s = x_main[s * BS : (s + 1) * BS].T  # [480, BS]
        xm_s = np.ascontiguousarray(
            xm_s.reshape(4, 120, BS).transpose(1, 0, 2)
        )  # [120, 4, BS]
        xg_s = np.ascontiguousarray(x_gate[s * BS : (s + 1) * BS].T)  # [128, BS]
        in_maps.append({**shared, "xm": xm_s, "xg": xg_s})

    global _last_in_maps
    _last_in_maps = in_maps
    res = run_bass_kernel_spmd(nc, in_maps, list(range(NCORES))).results
    return np.concatenate([res[s]["y"] for s in range(NCORES)], axis=0)


_last_in_maps = None



# revision 3
# speedup vs baseline: 1.1301x; 1.1301x over previous
"""Trainium2 Bass kernel for a soft-MoE (MANN) block.

Reference math (per token b):
    g  = elu(x_gate @ g1_w.T + g1_b); g = elu(g @ g2_w.T + g2_b)
    ew = softmax(g @ g3_w.T + g3_b)                      # [B, K=8]
    h1 = elu(sum_k ew_k * (x_main @ W1_k.T) + ew @ b1)   # [B, 1024]
    h2 = elu(sum_k ew_k * (h1 @ W2_k.T) + ew @ b2)       # [B, 1024]
    y  =     sum_k ew_k * (h2 @ W3_k.T) + ew @ b3        # [B, 640]

Strategy: data-parallel over 8 NeuronCores (128 batch rows per core),
expert weights replicated, streamed from HBM in bf16 (fp32 accumulate).
The per-expert combine is folded into PSUM accumulation by scaling the
layer *input* with ew_k before the matmul ("scale-before"), so each
output chunk is one PSUM accumulation group over (expert, i-tile).

The schedule is DMA-bound (35.1 MB of bf16 weights per core at 360 GB/s
aggregate), so the program is arranged to keep the DMA engines streaming
gaplessly: the first W1 expert load is issued before anything else on
the SP queue, the gating parameters are packed into a single small blob
issued on the Activation queue, W3 is split into (512|128) column slices
so the final PSUM groups close earlier, and the weight pools are deep
enough that buffer recycling never stalls the stream.
"""

import sys

sys.path.insert(0, "/opt/trn_rl_repo")

from contextlib import ExitStack

import numpy as np
import ml_dtypes

import concourse.bass as bass
from concourse import bacc
import concourse.tile as tile
from concourse import mybir
from concourse.bass_utils import run_bass_kernel_spmd
from concourse.masks import make_identity

F32 = mybir.dt.float32
BF16 = mybir.dt.bfloat16
AF = mybir.ActivationFunctionType
OP = mybir.AluOpType

B = 1024
X_MAIN, X_GATE, Y_DIM = 480, 128, 640
HID, GHID, K = 1024, 64, 8
NCORES = 8
BS = B // NCORES  # 128 batch rows per core

# packed gating-parameter blob column offsets (f32, [128, GP_COLS])
GP_G1W = 0          # [X_GATE=128, 64]
GP_G2W = 64         # [64, 64]
GP_G3W = 128        # [64, 8]
GP_G1B = 136        # [64, 1]
GP_G2B = 137        # [64, 1]
GP_G3B = 138        # [1, 8]
GP_COLS = 146

# trunk layer configs: (partition size of i-tiles, #i-tiles, O, o-chunk sizes)
L1 = (120, 4, HID, (512, 512))
L2 = (128, 8, HID, (512, 512))
L3 = (128, 8, Y_DIM, (512, 128))


def _build_program(with_bias: tuple[bool, bool, bool]) -> bass.Bass:
    nc = bacc.Bacc()

    # ---- DRAM parameters (host supplies exactly these layouts) ----
    xm_ext = nc.declare_dram_parameter("xm", [120, 4, BS], BF16, isOutput=False)
    xg_ext = nc.declare_dram_parameter("xg", [X_GATE, BS], F32, isOutput=False)
    gp_ext = nc.declare_dram_parameter("gp", [128, GP_COLS], F32, isOutput=False)
    w1_ext = nc.declare_dram_parameter("w1", [K, 120, 4, HID], BF16, isOutput=False)
    w2_ext = nc.declare_dram_parameter("w2", [K, 128, 8, HID], BF16, isOutput=False)
    w3a_ext = nc.declare_dram_parameter("w3a", [K, 128, 8 * 512], BF16, isOutput=False)
    w3b_ext = nc.declare_dram_parameter("w3b", [K, 128, 8 * 128], BF16, isOutput=False)
    b_ext = []
    for li, (P, IT, O, _) in enumerate((L1, L2, L3)):
        if with_bias[li]:
            b_ext.append(
                nc.declare_dram_parameter(f"b{li + 1}", [K, O], F32, isOutput=False)
            )
        else:
            b_ext.append(None)
    y_ext = nc.declare_dram_parameter("y", [BS, Y_DIM], BF16, isOutput=True)

    with TileCtx(nc) as tc, ExitStack() as ctx:
        const = ctx.enter_context(tc.tile_pool(name="const", bufs=1))
        gat = ctx.enter_context(tc.tile_pool(name="gat", bufs=1))
        spsum = ctx.enter_context(tc.tile_pool(name="spsum", bufs=2, space="PSUM"))
        zpsum = ctx.enter_context(tc.tile_pool(name="zpsum", bufs=3, space="PSUM"))
        tpsum = ctx.enter_context(tc.tile_pool(name="tpsum", bufs=2, space="PSUM"))
        xpool = ctx.enter_context(tc.tile_pool(name="xpool", bufs=1))
        xkp = ctx.enter_context(tc.tile_pool(name="xkp", bufs=2))
        hscr = ctx.enter_context(tc.tile_pool(name="hscr", bufs=1))
        hpool = ctx.enter_context(tc.tile_pool(name="hpool", bufs=2))
        w1p = ctx.enter_context(tc.tile_pool(name="w1p", bufs=4))
        w2p = ctx.enter_context(tc.tile_pool(name="w2p", bufs=4))
        w3ap = ctx.enter_context(tc.tile_pool(name="w3ap", bufs=4))
        w3bp = ctx.enter_context(tc.tile_pool(name="w3bp", bufs=4))

        # W1 expert 0 first: the weight stream owns the DMA engines from t=0.
        w1_tiles = [w1p.tile([120, 4, HID], BF16, tag="w1", name="w1_0")]
        nc.sync.dma_start(w1_tiles[0], w1_ext[0])

        # Small inputs ride the Activation HWDGE queue, slotting into the
        # stream behind W1-0 without delaying the SP weight queue.
        gp_sb = gat.tile([128, GP_COLS], F32)
        nc.scalar.dma_start(gp_sb, gp_ext[:])
        xg_sb = gat.tile([X_GATE, BS], F32)
        nc.scalar.dma_start(xg_sb, xg_ext[:])
        x1_sb = xpool.tile([120, 4, BS], BF16, tag="x1")
        nc.scalar.dma_start(x1_sb, xm_ext[:])

        identb = const.tile([128, 128], BF16)
        make_identity(nc, identb)
        onesb = const.tile([1, BS], BF16)
        nc.vector.memset(onesb, 1.0)

        # ---------------- gating (fp32) ----------------
        g1w_sb = gp_sb[:, GP_G1W : GP_G1W + GHID]
        g2w_sb = gp_sb[:GHID, GP_G2W : GP_G2W + GHID]
        g3w_sb = gp_sb[:GHID, GP_G3W : GP_G3W + K]
        g1b_sb = gp_sb[:GHID, GP_G1B : GP_G1B + 1]
        g2b_sb = gp_sb[:GHID, GP_G2B : GP_G2B + 1]
        g3b_sb = gp_sb[0:1, GP_G3B : GP_G3B + K]

        def gate_elup(zp, bias_sb, name):
            # returns elu(z + bias) + 1 = relu(z+bias) + exp(min(z+bias, 0)), [GHID, BS] f32
            r = gat.tile([GHID, BS], F32, tag=f"r_{name}")
            nc.scalar.activation(r, zp, AF.Relu, bias=bias_sb)
            m = gat.tile([GHID, BS], F32, tag=f"m_{name}")
            nc.vector.tensor_scalar(m, zp, bias_sb, 0.0, OP.add, OP.min)
            e = gat.tile([GHID, BS], F32, tag=f"e_{name}")
            nc.scalar.activation(e, m, AF.Exp)
            hp = gat.tile([GHID, BS], F32, tag=f"hp_{name}")
            nc.vector.tensor_tensor(hp, r, e, OP.add)
            return hp

        zg1 = spsum.tile([GHID, BS], F32, tag="g")
        nc.tensor.matmul(zg1, lhsT=g1w_sb, rhs=xg_sb, start=True, stop=True)
        h1p = gate_elup(zg1, g1b_sb, "g1")

        zg2 = spsum.tile([GHID, BS], F32, tag="g")
        nc.tensor.matmul(zg2, lhsT=g2w_sb, rhs=h1p, start=True, stop=True)
        h2p = gate_elup(zg2, g2b_sb, "g2")

        # logits in [b, k] layout: lhsT = h2p [GHID, BS], rhs = g3w [GHID, K]
        onesf = const.tile([1, BS], F32)
        nc.vector.memset(onesf, 1.0)
        zg3 = spsum.tile([BS, K], F32, tag="g")
        nc.tensor.matmul(zg3, lhsT=h2p, rhs=g3w_sb, start=True, stop=False)
        nc.tensor.matmul(zg3, lhsT=onesf, rhs=g3b_sb, start=False, stop=True)

        # softmax along free dim (K)
        negmx = gat.tile([BS, 1], F32)
        nc.vector.tensor_reduce(negmx, zg3, mybir.AxisListType.X, OP.max, negate=True)
        e3 = gat.tile([BS, K], F32)
        ssum = gat.tile([BS, 1], F32)
        nc.scalar.activation(e3, zg3, AF.Exp, bias=negmx[:, 0:1], accum_out=ssum[:, 0:1])
        rcp = gat.tile([BS, 1], F32)
        nc.vector.reciprocal(rcp, ssum)
        ewT = gat.tile([BS, K], BF16)  # [b, k]
        nc.vector.tensor_scalar_mul(ewT, e3, rcp[:, 0:1])

        # per-expert row at partition 0: ew_rows[0, k, :] = ewT[:, k].T
        ew_rows = gat.tile([1, K, BS], BF16)
        for k in range(K):
            rp = spsum.tile([1, BS], BF16, tag="g")
            nc.tensor.transpose(rp, ewT[:, k : k + 1], identb)
            nc.vector.tensor_copy(out=ew_rows[:, k, :], in_=rp)

        # broadcast rows: ewb[:, k, :] = ew_k replicated over all 128 partitions
        ewb = gat.tile([128, K, BS], BF16)
        for k in range(K):
            bp = spsum.tile([128, BS], F32, tag="g")
            nc.tensor.matmul(
                bp, lhsT=onesb, rhs=ew_rows[:, k, :], start=True, stop=True
            )
            nc.vector.tensor_copy(out=ewb[:, k, :], in_=bp)

        if any(with_bias):
            identf = const.tile([128, 128], F32)
            make_identity(nc, identf)
            ewTf = gat.tile([BS, K], F32)
            nc.vector.tensor_scalar_mul(ewTf, e3, rcp[:, 0:1])
            # ew [K, BS] on partitions 0..K-1 (lhsT for the bias matmuls)
            ewps = spsum.tile([K, BS], F32, tag="g")
            nc.tensor.transpose(ewps, ewTf, identf)
            ew_sb = gat.tile([K, BS], F32)
            nc.vector.tensor_copy(out=ew_sb, in_=ewps)

        # ---------------- trunk ----------------
        x_sb = x1_sb
        for li, (P, IT, O, chunks) in enumerate((L1, L2, L3)):
            last = li == 2
            # scale-before: xk[:, k, it, :] = x * ew_k  (bf16)
            xk = xkp.tile([P, K, IT, BS], BF16, tag="xk", name=f"xk{li}")
            for k in range(K):
                nc.vector.tensor_tensor(
                    xk[:, k],
                    x_sb,
                    ewb[:P, k, None, :].to_broadcast((P, IT, BS)),
                    OP.mult,
                )
            if not last:
                nx_sb = xpool.tile([128, O // 128, BS], BF16, tag=f"x{li + 2}")
            if b_ext[li] is not None:
                bl_sb = gat.tile([K, O], F32, tag=f"bias{li}")
                nc.sync.dma_start(bl_sb, b_ext[li][:])

            zps = []
            oc0 = 0
            for ci, ocsz in enumerate(chunks):
                zp = zpsum.tile([BS, 512], F32, tag="z", name=f"zp{li}_{ci}")[:, :ocsz]
                if b_ext[li] is not None:
                    nc.tensor.matmul(
                        zp, lhsT=ew_sb, rhs=bl_sb[:, oc0 : oc0 + ocsz],
                        start=True, stop=False,
                    )
                zps.append((zp, oc0, ocsz))
                oc0 += ocsz

            for k in range(K):
                if li == 0:
                    if k > 0:
                        w_sb = w1p.tile([120, 4, HID], BF16, tag="w1", name=f"w1_{k}")
                        nc.sync.dma_start(w_sb, w1_ext[k])
                    else:
                        w_sb = w1_tiles[0]
                    wslices = [
                        lambda it, occ, ocsz, w=w_sb: w[:, it, occ : occ + ocsz]
                    ]
                elif li == 1:
                    w_sb = w2p.tile([128, 8, HID], BF16, tag="w2", name=f"w2_{k}")
                    nc.sync.dma_start(w_sb, w2_ext[k])
                    wslices = [
                        lambda it, occ, ocsz, w=w_sb: w[:, it, occ : occ + ocsz]
                    ]
                else:
                    wa_sb = w3ap.tile([128, 8 * 512], BF16, tag="w3a", name=f"w3a_{k}")
                    nc.sync.dma_start(wa_sb, w3a_ext[k])
                    wb_sb = w3bp.tile([128, 8 * 128], BF16, tag="w3b", name=f"w3b_{k}")
                    nc.sync.dma_start(wb_sb, w3b_ext[k])
                    wslices = [
                        lambda it, occ, ocsz, w=wa_sb: w[:, it * 512 : it * 512 + ocsz],
                        lambda it, occ, ocsz, w=wb_sb: w[
                            :, it * 128 : it * 128 + ocsz
                        ],
                    ]
                for ci, (zp, occ, ocsz) in enumerate(zps):
                    wsl = wslices[min(ci, len(wslices) - 1)]
                    for it in range(IT):
                        nc.tensor.matmul(
                            zp,
                            lhsT=xk[:, k, it, :],
                            rhs=wsl(it, occ, ocsz),
                            start=(k == 0 and it == 0 and b_ext[li] is None),
                            stop=(k == K - 1 and it == IT - 1),
                        )

            for zp, oc0, ocsz in zps:
                if last:
                    y_sb = hpool.tile([BS, 512], BF16, tag="y", name="y_sb")[:, :ocsz]
                    nc.vector.tensor_copy(out=y_sb, in_=zp)
                    nc.scalar.dma_start(y_ext[:, oc0 : oc0 + ocsz], y_sb)
                else:
                    # h = (max(z,0) - 1) + exp(min(z,0))   (= elu(z))
                    m = hscr.tile([BS, 512], F32, tag="hm", name="hm")[:, :ocsz]
                    nc.vector.tensor_scalar(m, zp, 0.0, None, OP.min)
                    e = hscr.tile([BS, 512], F32, tag="he", name="he")[:, :ocsz]
                    nc.scalar.activation(e, m, AF.Exp)
                    r = hscr.tile([BS, 512], F32, tag="hr", name="hr")[:, :ocsz]
                    nc.vector.tensor_scalar(r, zp, 0.0, -1.0, OP.max, OP.add)
                    h = hpool.tile([BS, 512], BF16, tag="hh", name="hh")[:, :ocsz]
                    nc.vector.tensor_tensor(h, r, e, OP.add)
                    # transpose each 128-col block into next layer's input layout
                    for j in range(ocsz // 128):
                        tp = tpsum.tile([128, BS], BF16, tag="tr")
                        nc.tensor.transpose(tp, h[:, j * 128 : (j + 1) * 128], identb)
                        nc.vector.tensor_copy(
                            out=nx_sb[:, (oc0 // 128) + j, :], in_=tp
                        )
            if not last:
                x_sb = nx_sb

    nc.compile()
    return nc


def TileCtx(nc):
    return tile.TileContext(nc)


_PROG_CACHE: dict = {}


def _get_program(with_bias):
    key = tuple(with_bias)
    if key not in _PROG_CACHE:
        _PROG_CACHE[key] = _build_program(key)
    return _PROG_CACHE[key]


def _prep_w(W, P, IT):
    # [K, O, I] -> [K, P, IT, O] with element [k,p,it,o] = W[k,o,it*P+p]
    Kk, O, I = W.shape
    Wt = W.transpose(0, 2, 1).reshape(Kk, IT, P, O).transpose(0, 2, 1, 3)
    return np.ascontiguousarray(Wt.astype(ml_dtypes.bfloat16))


def kernel(
    x_main, x_gate, g1_w, g1_b, g2_w, g2_b, g3_w, g3_b,
    W1, b1, W2, b2, W3, b3,
):
    x_main = np.asarray(x_main, np.float32)
    x_gate = np.asarray(x_gate, np.float32)
    g1_w = np.asarray(g1_w, np.float32)
    g1_b = np.asarray(g1_b, np.float32)
    g2_w = np.asarray(g2_w, np.float32)
    g2_b = np.asarray(g2_b, np.float32)
    g3_w = np.asarray(g3_w, np.float32)
    g3_b = np.asarray(g3_b, np.float32)
    W1 = np.asarray(W1, np.float32)
    b1 = np.asarray(b1, np.float32)
    W2 = np.asarray(W2, np.float32)
    b2 = np.asarray(b2, np.float32)
    W3 = np.asarray(W3, np.float32)
    b3 = np.asarray(b3, np.float32)

    with_bias = (bool(b1.any()), bool(b2.any()), bool(b3.any()))
    nc = _get_program(with_bias)

    gp = np.zeros((128, GP_COLS), np.float32)
    gp[:, GP_G1W : GP_G1W + GHID] = g1_w.T
    gp[:GHID, GP_G2W : GP_G2W + GHID] = g2_w.T
    gp[:GHID, GP_G3W : GP_G3W + K] = g3_w.T
    gp[:GHID, GP_G1B] = g1_b
    gp[:GHID, GP_G2B] = g2_b - g2_w.sum(1)
    gp[0, GP_G3B : GP_G3B + K] = g3_b - g3_w.sum(1)

    w3 = _prep_w(W3, 128, 8)  # [K, 128, 8, 640]
    shared = {
        "gp": gp,
        "w1": _prep_w(W1, 120, 4),
        "w2": _prep_w(W2, 128, 8),
        "w3a": np.ascontiguousarray(w3[:, :, :, 0:512].reshape(K, 128, 8 * 512)),
        "w3b": np.ascontiguousarray(w3[:, :, :, 512:640].reshape(K, 128, 8 * 128)),
    }
    for name, b, flag in (("b1", b1, with_bias[0]), ("b2", b2, with_bias[1]),
                          ("b3", b3, with_bias[2])):
        if flag:
            shared[name] = np.ascontiguousarray(b)

    in_maps = []
    for s in range(NCORES):
        xm_s = x_main[s * BS : (s + 1) * BS].T  # [480, BS]
        xm_s = np.ascontiguousarray(
            xm_s.reshape(4, 120, BS).transpose(1, 0, 2).astype(ml_dtypes.bfloat16)
        )  # [120, 4, BS] bf16
        xg_s = np.ascontiguousarray(x_gate[s * BS : (s + 1) * BS].T)  # [128, BS]
        in_maps.append({**shared, "xm": xm_s, "xg": xg_s})

    global _last_in_maps
    _last_in_maps = in_maps
    res = run_bass_kernel_spmd(nc, in_maps, list(range(NCORES))).results
    return np.concatenate(
        [np.asarray(res[s]["y"]).astype(np.float32) for s in range(NCORES)], axis=0
    )


_last_in_maps = None


# revision 8
# speedup vs baseline: 1.1596x; 1.0261x over previous
"""Trainium2 Bass kernel for a soft-MoE (MANN) block.

Reference math (per token b):
    g  = elu(x_gate @ g1_w.T + g1_b); g = elu(g @ g2_w.T + g2_b)
    ew = softmax(g @ g3_w.T + g3_b)                      # [B, K=8]
    h1 = elu(sum_k ew_k * (x_main @ W1_k.T) + ew @ b1)   # [B, 1024]
    h2 = elu(sum_k ew_k * (h1 @ W2_k.T) + ew @ b2)       # [B, 1024]
    y  =     sum_k ew_k * (h2 @ W3_k.T) + ew @ b3        # [B, 640]

Strategy: data-parallel over 8 NeuronCores (128 batch rows per core),
expert weights replicated, streamed from HBM in bf16 (fp32 accumulate).
The per-expert combine is folded into PSUM accumulation by scaling the
layer *input* with ew_k before the matmul ("scale-before"), so each
output chunk is one PSUM accumulation group over (expert, i-tile).

The schedule is DMA-bound (35.1 MB of bf16 weights per core at 360 GB/s
aggregate), so the program is arranged to keep the DMA engines streaming
gaplessly: the first W1 expert load is issued before anything else on
the SP queue, the gating parameters are packed into a single small blob
issued on the Activation queue, W3 is split into (512|128) column slices
so the final PSUM groups close earlier, and the weight pools are deep
enough that buffer recycling never stalls the stream.
"""

import sys

sys.path.insert(0, "/opt/trn_rl_repo")

from contextlib import ExitStack

import numpy as np
import ml_dtypes

import concourse.bass as bass
from concourse import bacc
import concourse.tile as tile
from concourse import mybir
from concourse.bass_utils import run_bass_kernel_spmd
from concourse.masks import make_identity

F32 = mybir.dt.float32
BF16 = mybir.dt.bfloat16
AF = mybir.ActivationFunctionType
OP = mybir.AluOpType

B = 1024
X_MAIN, X_GATE, Y_DIM = 480, 128, 640
HID, GHID, K = 1024, 64, 8
NCORES = 8
BS = B // NCORES  # 128 batch rows per core

# packed gating-parameter blob column offsets (f32, [128, GP_COLS])
GP_G1W = 0          # [X_GATE=128, 64]
GP_G2W = 64         # [64, 64]
GP_G3W = 128        # [64, 8]
GP_G1B = 136        # [64, 1]
GP_G2B = 137        # [64, 1]
GP_G3B = 138        # [1, 8]
GP_COLS = 146

# trunk layer configs: (partition size of i-tiles, #i-tiles, O, o-chunk sizes)
L1 = (120, 4, HID, (512, 512))
L2 = (128, 8, HID, (512, 512))
L3 = (128, 8, Y_DIM, (256, 256, 128))


def _build_program(with_bias: tuple[bool, bool, bool]) -> bass.Bass:
    nc = bacc.Bacc()

    # ---- DRAM parameters (host supplies exactly these layouts) ----
    xm_ext = nc.declare_dram_parameter("xm", [120, 4, BS], BF16, isOutput=False)
    xg_ext = nc.declare_dram_parameter("xg", [X_GATE, BS], F32, isOutput=False)
    gp_ext = nc.declare_dram_parameter("gp", [128, GP_COLS], F32, isOutput=False)
    w1_ext = nc.declare_dram_parameter("w1", [K, 120, 4, HID], BF16, isOutput=False)
    w2_ext = nc.declare_dram_parameter("w2", [K, 128, 8, HID], BF16, isOutput=False)
    w3_ext = [
        nc.declare_dram_parameter(f"w3{c}", [K, 128, 8 * sz], BF16, isOutput=False)
        for c, sz in zip("abc", L3[3])
    ]
    b_ext = []
    for li, (P, IT, O, _) in enumerate((L1, L2, L3)):
        if with_bias[li]:
            b_ext.append(
                nc.declare_dram_parameter(f"b{li + 1}", [K, O], F32, isOutput=False)
            )
        else:
            b_ext.append(None)
    y_ext = nc.declare_dram_parameter("y", [BS, Y_DIM], BF16, isOutput=True)

    with TileCtx(nc) as tc, ExitStack() as ctx:
        const = ctx.enter_context(tc.tile_pool(name="const", bufs=1))
        gat = ctx.enter_context(tc.tile_pool(name="gat", bufs=1))
        spsum = ctx.enter_context(tc.tile_pool(name="spsum", bufs=2, space="PSUM"))
        zpsum = ctx.enter_context(tc.tile_pool(name="zpsum", bufs=3, space="PSUM"))
        tpsum = ctx.enter_context(tc.tile_pool(name="tpsum", bufs=2, space="PSUM"))
        xpool = ctx.enter_context(tc.tile_pool(name="xpool", bufs=1))
        xkp = ctx.enter_context(tc.tile_pool(name="xkp", bufs=2))
        hscr = ctx.enter_context(tc.tile_pool(name="hscr", bufs=1))
        hpool = ctx.enter_context(tc.tile_pool(name="hpool", bufs=2))
        w1p = ctx.enter_context(tc.tile_pool(name="w1p", bufs=4))
        w2p = ctx.enter_context(tc.tile_pool(name="w2p", bufs=4))
        w3p = [
            ctx.enter_context(tc.tile_pool(name=f"w3{c}p", bufs=4)) for c in "abc"
        ]

        # Small inputs first on the SP queue (~0.7us), then the weight
        # stream; total stream length is order-invariant but this gets the
        # gating chain started ~5us earlier, so L1 matmuls free the w1p
        # buffers before the stream needs them back.
        xg_sb = gat.tile([X_GATE, BS], F32)
        nc.sync.dma_start(xg_sb, xg_ext[:])
        x1_sb = xpool.tile([120, 4, BS], BF16, tag="x1")
        nc.sync.dma_start(x1_sb, xm_ext[:])
        gp_sb = gat.tile([128, GP_COLS], F32)
        nc.sync.dma_start(gp_sb, gp_ext[:])

        w1_tiles = [w1p.tile([120, 4, HID], BF16, tag="w1", name="w1_0")]
        nc.sync.dma_start(w1_tiles[0], w1_ext[0])

        identb = const.tile([128, 128], BF16)
        make_identity(nc, identb)
        onesb = const.tile([1, BS], BF16)
        nc.vector.memset(onesb, 1.0)

        # ---------------- gating (fp32) ----------------
        g1w_sb = gp_sb[:, GP_G1W : GP_G1W + GHID]
        g2w_sb = gp_sb[:GHID, GP_G2W : GP_G2W + GHID]
        g3w_sb = gp_sb[:GHID, GP_G3W : GP_G3W + K]
        g1b_sb = gp_sb[:GHID, GP_G1B : GP_G1B + 1]
        g2b_sb = gp_sb[:GHID, GP_G2B : GP_G2B + 1]
        g3b_sb = gp_sb[0:1, GP_G3B : GP_G3B + K]

        def gate_elup(zp, bias_sb, name):
            # returns elu(z + bias) + 1 = relu(z+bias) + exp(min(z+bias, 0)), [GHID, BS] f32
            r = gat.tile([GHID, BS], F32, tag=f"r_{name}")
            nc.scalar.activation(r, zp, AF.Relu, bias=bias_sb)
            m = gat.tile([GHID, BS], F32, tag=f"m_{name}")
            nc.vector.tensor_scalar(m, zp, bias_sb, 0.0, OP.add, OP.min)
            e = gat.tile([GHID, BS], F32, tag=f"e_{name}")
            nc.scalar.activation(e, m, AF.Exp)
            hp = gat.tile([GHID, BS], F32, tag=f"hp_{name}")
            nc.vector.tensor_tensor(hp, r, e, OP.add)
            return hp

        zg1 = spsum.tile([GHID, BS], F32, tag="g")
        nc.tensor.matmul(zg1, lhsT=g1w_sb, rhs=xg_sb, start=True, stop=True)
        h1p = gate_elup(zg1, g1b_sb, "g1")

        zg2 = spsum.tile([GHID, BS], F32, tag="g")
        nc.tensor.matmul(zg2, lhsT=g2w_sb, rhs=h1p, start=True, stop=True)
        h2p = gate_elup(zg2, g2b_sb, "g2")

        # logits in [b, k] layout: lhsT = h2p [GHID, BS], rhs = g3w [GHID, K]
        onesf = const.tile([1, BS], F32)
        nc.vector.memset(onesf, 1.0)
        zg3 = spsum.tile([BS, K], F32, tag="g")
        nc.tensor.matmul(zg3, lhsT=h2p, rhs=g3w_sb, start=True, stop=False)
        nc.tensor.matmul(zg3, lhsT=onesf, rhs=g3b_sb, start=False, stop=True)

        # softmax along free dim (K)
        negmx = gat.tile([BS, 1], F32)
        nc.vector.tensor_reduce(negmx, zg3, mybir.AxisListType.X, OP.max, negate=True)
        e3 = gat.tile([BS, K], F32)
        ssum = gat.tile([BS, 1], F32)
        nc.scalar.activation(e3, zg3, AF.Exp, bias=negmx[:, 0:1], accum_out=ssum[:, 0:1])
        rcp = gat.tile([BS, 1], F32)
        nc.vector.reciprocal(rcp, ssum)
        ewT = gat.tile([BS, K], BF16)  # [b, k]
        nc.vector.tensor_scalar_mul(ewT, e3, rcp[:, 0:1])

        # per-expert row at partition 0: ew_rows[0, k, :] = ewT[:, k].T
        ew_rows = gat.tile([1, K, BS], BF16)
        for k in range(K):
            rp = spsum.tile([1, BS], BF16, tag="g")
            nc.tensor.transpose(rp, ewT[:, k : k + 1], identb)
            nc.vector.tensor_copy(out=ew_rows[:, k, :], in_=rp)

        # broadcast rows: ewb[:, k, :] = ew_k replicated over all 128 partitions
        ewb = gat.tile([128, K, BS], BF16)
        for k in range(K):
            bp = spsum.tile([128, BS], F32, tag="g")
            nc.tensor.matmul(
                bp, lhsT=onesb, rhs=ew_rows[:, k, :], start=True, stop=True
            )
            nc.vector.tensor_copy(out=ewb[:, k, :], in_=bp)

        if any(with_bias):
            identf = const.tile([128, 128], F32)
            make_identity(nc, identf)
            ewTf = gat.tile([BS, K], F32)
            nc.vector.tensor_scalar_mul(ewTf, e3, rcp[:, 0:1])
            # ew [K, BS] on partitions 0..K-1 (lhsT for the bias matmuls)
            ewps = spsum.tile([K, BS], F32, tag="g")
            nc.tensor.transpose(ewps, ewTf, identf)
            ew_sb = gat.tile([K, BS], F32)
            nc.vector.tensor_copy(out=ew_sb, in_=ewps)

        # ---------------- trunk ----------------
        x_sb = x1_sb
        for li, (P, IT, O, chunks) in enumerate((L1, L2, L3)):
            last = li == 2
            # scale-before: xk[:, k, it, :] = x * ew_k  (bf16)
            xk = xkp.tile([P, K, IT, BS], BF16, tag="xk", name=f"xk{li}")
            for k in range(K):
                nc.vector.tensor_tensor(
                    xk[:, k],
                    x_sb,
                    ewb[:P, k, None, :].to_broadcast((P, IT, BS)),
                    OP.mult,
                )
            if not last:
                nx_sb = xpool.tile([128, O // 128, BS], BF16, tag=f"x{li + 2}")
            if b_ext[li] is not None:
                bl_sb = gat.tile([K, O], F32, tag=f"bias{li}")
                nc.sync.dma_start(bl_sb, b_ext[li][:])

            zps = []
            oc0 = 0
            for ci, ocsz in enumerate(chunks):
                zp = zpsum.tile([BS, 512], F32, tag="z", name=f"zp{li}_{ci}")[:, :ocsz]
                if b_ext[li] is not None:
                    nc.tensor.matmul(
                        zp, lhsT=ew_sb, rhs=bl_sb[:, oc0 : oc0 + ocsz],
                        start=True, stop=False,
                    )
                zps.append((zp, oc0, ocsz))
                oc0 += ocsz

            for k in range(K):
                if li == 0:
                    if k > 0:
                        w_sb = w1p.tile([120, 4, HID], BF16, tag="w1", name=f"w1_{k}")
                        nc.sync.dma_start(w_sb, w1_ext[k])
                    else:
                        w_sb = w1_tiles[0]
                    wslices = [
                        lambda it, occ, ocsz, w=w_sb: w[:, it, occ : occ + ocsz]
                    ]
                elif li == 1:
                    w_sb = w2p.tile([128, 8, HID], BF16, tag="w2", name=f"w2_{k}")
                    nc.sync.dma_start(w_sb, w2_ext[k])
                    wslices = [
                        lambda it, occ, ocsz, w=w_sb: w[:, it, occ : occ + ocsz]
                    ]
                else:
                    wslices = []
                    for ci, csz in enumerate(chunks):
                        wc_sb = w3p[ci].tile(
                            [128, 8 * csz], BF16, tag=f"w3{ci}", name=f"w3{ci}_{k}"
                        )
                        nc.sync.dma_start(wc_sb, w3_ext[ci][k])
                        wslices.append(
                            lambda it, occ, ocsz, w=wc_sb, c=csz: w[
                                :, it * c : it * c + ocsz
                            ]
                        )
                for ci, (zp, occ, ocsz) in enumerate(zps):
                    wsl = wslices[min(ci, len(wslices) - 1)]
                    for it in range(IT):
                        nc.tensor.matmul(
                            zp,
                            lhsT=xk[:, k, it, :],
                            rhs=wsl(it, occ, ocsz),
                            start=(k == 0 and it == 0 and b_ext[li] is None),
                            stop=(k == K - 1 and it == IT - 1),
                        )

            for zp, oc0, ocsz in zps:
                if last:
                    y_sb = hpool.tile([BS, 512], BF16, tag="y", name="y_sb")[:, :ocsz]
                    nc.vector.tensor_copy(out=y_sb, in_=zp)
                    nc.scalar.dma_start(y_ext[:, oc0 : oc0 + ocsz], y_sb)
                else:
                    # h = (max(z,0) - 1) + exp(min(z,0))   (= elu(z))
                    m = hscr.tile([BS, 512], F32, tag="hm", name="hm")[:, :ocsz]
                    nc.vector.tensor_scalar(m, zp, 0.0, None, OP.min)
                    e = hscr.tile([BS, 512], F32, tag="he", name="he")[:, :ocsz]
                    nc.scalar.activation(e, m, AF.Exp)
                    r = hscr.tile([BS, 512], F32, tag="hr", name="hr")[:, :ocsz]
                    nc.vector.tensor_scalar(r, zp, 0.0, -1.0, OP.max, OP.add)
                    h = hpool.tile([BS, 512], BF16, tag="hh", name="hh")[:, :ocsz]
                    nc.vector.tensor_tensor(h, r, e, OP.add)
                    # transpose each 128-col block into next layer's input layout
                    for j in range(ocsz // 128):
                        tp = tpsum.tile([128, BS], BF16, tag="tr")
                        nc.tensor.transpose(tp, h[:, j * 128 : (j + 1) * 128], identb)
                        nc.vector.tensor_copy(
                            out=nx_sb[:, (oc0 // 128) + j, :], in_=tp
                        )
            if not last:
                x_sb = nx_sb

    nc.compile()
    return nc


def TileCtx(nc):
    return tile.TileContext(nc)


_PROG_CACHE: dict = {}


def _get_program(with_bias):
    key = tuple(with_bias)
    if key not in _PROG_CACHE:
        _PROG_CACHE[key] = _build_program(key)
    return _PROG_CACHE[key]


def _prep_w(W, P, IT):
    # [K, O, I] -> [K, P, IT, O] with element [k,p,it,o] = W[k,o,it*P+p]
    Kk, O, I = W.shape
    Wt = W.transpose(0, 2, 1).reshape(Kk, IT, P, O).transpose(0, 2, 1, 3)
    return np.ascontiguousarray(Wt.astype(ml_dtypes.bfloat16))


def kernel(
    x_main, x_gate, g1_w, g1_b, g2_w, g2_b, g3_w, g3_b,
    W1, b1, W2, b2, W3, b3,
):
    x_main = np.asarray(x_main, np.float32)
    x_gate = np.asarray(x_gate, np.float32)
    g1_w = np.asarray(g1_w, np.float32)
    g1_b = np.asarray(g1_b, np.float32)
    g2_w = np.asarray(g2_w, np.float32)
    g2_b = np.asarray(g2_b, np.float32)
    g3_w = np.asarray(g3_w, np.float32)
    g3_b = np.asarray(g3_b, np.float32)
    W1 = np.asarray(W1, np.float32)
    b1 = np.asarray(b1, np.float32)
    W2 = np.asarray(W2, np.float32)
    b2 = np.asarray(b2, np.float32)
    W3 = np.asarray(W3, np.float32)
    b3 = np.asarray(b3, np.float32)

    with_bias = (bool(b1.any()), bool(b2.any()), bool(b3.any()))
    nc = _get_program(with_bias)

    gp = np.zeros((128, GP_COLS), np.float32)
    gp[:, GP_G1W : GP_G1W + GHID] = g1_w.T
    gp[:GHID, GP_G2W : GP_G2W + GHID] = g2_w.T
    gp[:GHID, GP_G3W : GP_G3W + K] = g3_w.T
    gp[:GHID, GP_G1B] = g1_b
    gp[:GHID, GP_G2B] = g2_b - g2_w.sum(1)
    gp[0, GP_G3B : GP_G3B + K] = g3_b - g3_w.sum(1)

    w3 = _prep_w(W3, 128, 8)  # [K, 128, 8, 640]
    shared = {
        "gp": gp,
        "w1": _prep_w(W1, 120, 4),
        "w2": _prep_w(W2, 128, 8),
    }
    oc0 = 0
    for c, sz in zip("abc", L3[3]):
        shared[f"w3{c}"] = np.ascontiguousarray(
            w3[:, :, :, oc0 : oc0 + sz].reshape(K, 128, 8 * sz)
        )
        oc0 += sz
    for name, b, flag in (("b1", b1, with_bias[0]), ("b2", b2, with_bias[1]),
                          ("b3", b3, with_bias[2])):
        if flag:
            shared[name] = np.ascontiguousarray(b)

    in_maps = []
    for s in range(NCORES):
        xm_s = x_main[s * BS : (s + 1) * BS].T  # [480, BS]
        xm_s = np.ascontiguousarray(
            xm_s.reshape(4, 120, BS).transpose(1, 0, 2).astype(ml_dtypes.bfloat16)
        )  # [120, 4, BS] bf16
        xg_s = np.ascontiguousarray(x_gate[s * BS : (s + 1) * BS].T)  # [128, BS]
        in_maps.append({**shared, "xm": xm_s, "xg": xg_s})

    global _last_in_maps
    _last_in_maps = in_maps
    res = run_bass_kernel_spmd(nc, in_maps, list(range(NCORES))).results
    return np.concatenate(
        [np.asarray(res[s]["y"]).astype(np.float32) for s in range(NCORES)], axis=0
    )


_last_in_maps = None


# revision 10
# speedup vs baseline: 1.1636x; 1.0035x over previous
"""Trainium2 Bass kernel for a soft-MoE (MANN) block.

Reference math (per token b):
    g  = elu(x_gate @ g1_w.T + g1_b); g = elu(g @ g2_w.T + g2_b)
    ew = softmax(g @ g3_w.T + g3_b)                      # [B, K=8]
    h1 = elu(sum_k ew_k * (x_main @ W1_k.T) + ew @ b1)   # [B, 1024]
    h2 = elu(sum_k ew_k * (h1 @ W2_k.T) + ew @ b2)       # [B, 1024]
    y  =     sum_k ew_k * (h2 @ W3_k.T) + ew @ b3        # [B, 640]

Strategy: data-parallel over 8 NeuronCores (128 batch rows per core),
expert weights replicated, streamed from HBM in bf16 (fp32 accumulate).
The per-expert combine is folded into PSUM accumulation by scaling the
layer *input* with ew_k before the matmul ("scale-before"), so each
output chunk is one PSUM accumulation group over (expert, i-tile).

The schedule is DMA-bound (35.1 MB of bf16 weights per core at 360 GB/s
aggregate), so the program is arranged to keep the DMA engines streaming
gaplessly from ~2us to the end:
  - all small inputs ride in ONE packed f32 blob DMA ahead of the
    weight stream (no issue-latency bubbles between tiny transfers);
  - per-expert xk scale tiles are separate tiles, built on demand, so
    each expert's matmuls unblock individually;
  - W3 is split column-wise into (256,256,128) chunk tensors and
    streamed chunk-major (all experts' chunk-a first), with the L3
    matmul loop also chunk-major, so the last expert's big-chunk
    matmuls run during the stream and only a 128-col chunk trails it;
  - weight pools are deep enough that buffer recycling never stalls
    the stream (W3 chunk-a is fully resident: 8 bufs).
"""

import sys

sys.path.insert(0, "/opt/trn_rl_repo")

from contextlib import ExitStack

import numpy as np
import ml_dtypes

import concourse.bass as bass
from concourse import bacc
import concourse.tile as tile
from concourse import mybir
from concourse.bass_utils import run_bass_kernel_spmd
from concourse.masks import make_identity

F32 = mybir.dt.float32
BF16 = mybir.dt.bfloat16
AF = mybir.ActivationFunctionType
OP = mybir.AluOpType

B = 1024
X_MAIN, X_GATE, Y_DIM = 480, 128, 640
HID, GHID, K = 1024, 64, 8
NCORES = 8
BS = B // NCORES  # 128 batch rows per core

# packed input blob: [128, 7, BS] f32.  Groups 0..3 = x_main i-tiles
# (rows 0:120), group 4 = x_gate, groups 5..6 = gating params:
#   [:,5,0:64]  g1w      [0:64,5,64:128] g2w
#   [0:64,6,0:8] g3w     [0:64,6,8]  g1b
#   [0:64,6,9]  g2b'     [0:1,6,16:24] g3b'
IB_GROUPS = 7

# trunk layer configs: (partition size of i-tiles, #i-tiles, O, o-chunk sizes)
L1 = (120, 4, HID, (512, 512))
L2 = (128, 8, HID, (512, 512))
L3 = (128, 8, Y_DIM, (256, 256, 128))


def _build_program(with_bias: tuple[bool, bool, bool]) -> bass.Bass:
    nc = bacc.Bacc()

    # ---- DRAM parameters (host supplies exactly these layouts) ----
    inp_ext = nc.declare_dram_parameter("inp", [128, IB_GROUPS, BS], F32, isOutput=False)
    w1_ext = nc.declare_dram_parameter("w1", [K, 120, 4, HID], BF16, isOutput=False)
    w2_ext = nc.declare_dram_parameter("w2", [K, 128, 8, HID], BF16, isOutput=False)
    w3_ext = [
        nc.declare_dram_parameter(f"w3{c}", [K, 128, 8 * sz], BF16, isOutput=False)
        for c, sz in zip("abc", L3[3])
    ]
    b_ext = []
    for li, (P, IT, O, _) in enumerate((L1, L2, L3)):
        if with_bias[li]:
            b_ext.append(
                nc.declare_dram_parameter(f"b{li + 1}", [K, O], F32, isOutput=False)
            )
        else:
            b_ext.append(None)
    y_ext = nc.declare_dram_parameter("y", [BS, Y_DIM], BF16, isOutput=True)

    with TileCtx(nc) as tc, ExitStack() as ctx:
        const = ctx.enter_context(tc.tile_pool(name="const", bufs=1))
        gat = ctx.enter_context(tc.tile_pool(name="gat", bufs=1))
        inpp = ctx.enter_context(tc.tile_pool(name="inpp", bufs=1))
        spsum = ctx.enter_context(tc.tile_pool(name="spsum", bufs=2, space="PSUM"))
        zpsum = ctx.enter_context(tc.tile_pool(name="zpsum", bufs=3, space="PSUM"))
        tpsum = ctx.enter_context(tc.tile_pool(name="tpsum", bufs=2, space="PSUM"))
        xpool = ctx.enter_context(tc.tile_pool(name="xpool", bufs=1))
        xkp = ctx.enter_context(tc.tile_pool(name="xkp", bufs=16))
        hscr = ctx.enter_context(tc.tile_pool(name="hscr", bufs=1))
        hpool = ctx.enter_context(tc.tile_pool(name="hpool", bufs=2))
        w1p = ctx.enter_context(tc.tile_pool(name="w1p", bufs=4))
        w2p = ctx.enter_context(tc.tile_pool(name="w2p", bufs=3))
        w3p = [
            ctx.enter_context(tc.tile_pool(name=f"w3{c}p", bufs=nb))
            for c, nb in zip("abc", (8, 4, 4))
        ]

        # One packed input DMA, then the weight stream (SP queue throughout).
        inp_sb = inpp.tile([128, IB_GROUPS, BS], F32)
        nc.sync.dma_start(inp_sb, inp_ext[:])

        w1_tiles = [w1p.tile([120, 4, HID], BF16, tag="w1", name="w1_0")]
        nc.sync.dma_start(w1_tiles[0], w1_ext[0])

        x1_sb = inp_sb[0:120, 0:4, :]
        xg_sb = inp_sb[:, 4, :]
        g1w_sb = inp_sb[:, 5, 0:GHID]
        g2w_sb = inp_sb[0:GHID, 5, GHID : 2 * GHID]
        g3w_sb = inp_sb[0:GHID, 6, 0:K]
        g1b_sb = inp_sb[0:GHID, 6, K : K + 1]
        g2b_sb = inp_sb[0:GHID, 6, K + 1 : K + 2]
        g3b_sb = inp_sb[0:1, 6, 16 : 16 + K]

        identb = const.tile([128, 128], BF16)
        make_identity(nc, identb)
        onesb = const.tile([1, BS], BF16)
        nc.vector.memset(onesb, 1.0)
        onesf = const.tile([1, BS], F32)
        nc.vector.memset(onesf, 1.0)

        # ---------------- gating (fp32) ----------------
        def gate_elup(zp, bias_sb, name):
            # returns elu(z + bias) + 1 = relu(z+bias) + exp(min(z+bias, 0)), [GHID, BS] f32
            r = gat.tile([GHID, BS], F32, tag=f"r_{name}")
            nc.scalar.activation(r, zp, AF.Relu, bias=bias_sb)
            m = gat.tile([GHID, BS], F32, tag=f"m_{name}")
            nc.vector.tensor_scalar(m, zp, bias_sb, 0.0, OP.add, OP.min)
            e = gat.tile([GHID, BS], F32, tag=f"e_{name}")
            nc.scalar.activation(e, m, AF.Exp)
            hp = gat.tile([GHID, BS], F32, tag=f"hp_{name}")
            nc.vector.tensor_tensor(hp, r, e, OP.add)
            return hp

        zg1 = spsum.tile([GHID, BS], F32, tag="g")
        nc.tensor.matmul(zg1, lhsT=g1w_sb, rhs=xg_sb, start=True, stop=True)
        h1p = gate_elup(zg1, g1b_sb, "g1")

        zg2 = spsum.tile([GHID, BS], F32, tag="g")
        nc.tensor.matmul(zg2, lhsT=g2w_sb, rhs=h1p, start=True, stop=True)
        h2p = gate_elup(zg2, g2b_sb, "g2")

        # logits in [b, k] layout: lhsT = h2p [GHID, BS], rhs = g3w [GHID, K]
        zg3 = spsum.tile([BS, K], F32, tag="g")
        nc.tensor.matmul(zg3, lhsT=h2p, rhs=g3w_sb, start=True, stop=False)
        nc.tensor.matmul(zg3, lhsT=onesf, rhs=g3b_sb, start=False, stop=True)

        # softmax along free dim (K)
        negmx = gat.tile([BS, 1], F32)
        nc.vector.tensor_reduce(negmx, zg3, mybir.AxisListType.X, OP.max, negate=True)
        e3 = gat.tile([BS, K], F32)
        ssum = gat.tile([BS, 1], F32)
        nc.scalar.activation(e3, zg3, AF.Exp, bias=negmx[:, 0:1], accum_out=ssum[:, 0:1])
        rcp = gat.tile([BS, 1], F32)
        nc.vector.reciprocal(rcp, ssum)
        ewT = gat.tile([BS, K], BF16)  # [b, k]
        nc.vector.tensor_scalar_mul(ewT, e3, rcp[:, 0:1])

        # per-expert row at partition 0: ew_rows[0, k, :] = ewT[:, k].T
        ew_rows = gat.tile([1, K, BS], BF16)
        for k in range(K):
            rp = spsum.tile([1, BS], BF16, tag="g")
            nc.tensor.transpose(rp, ewT[:, k : k + 1], identb)
            nc.vector.tensor_copy(out=ew_rows[:, k, :], in_=rp)

        # broadcast rows: ewb[:, k, :] = ew_k replicated over all 128 partitions
        ewb = gat.tile([128, K, BS], BF16)
        for k in range(K):
            bp = spsum.tile([128, BS], F32, tag="g")
            nc.tensor.matmul(
                bp, lhsT=onesb, rhs=ew_rows[:, k, :], start=True, stop=True
            )
            nc.vector.tensor_copy(out=ewb[:, k, :], in_=bp)

        if any(with_bias):
            identf = const.tile([128, 128], F32)
            make_identity(nc, identf)
            ewTf = gat.tile([BS, K], F32)
            nc.vector.tensor_scalar_mul(ewTf, e3, rcp[:, 0:1])
            # ew [K, BS] on partitions 0..K-1 (lhsT for the bias matmuls)
            ewps = spsum.tile([K, BS], F32, tag="g")
            nc.tensor.transpose(ewps, ewTf, identf)
            ew_sb = gat.tile([K, BS], F32)
            nc.vector.tensor_copy(out=ew_sb, in_=ewps)

        # ---------------- trunk ----------------
        def make_xk(x_sb, P, IT, k, li):
            xk = xkp.tile([P, IT, BS], BF16, tag="xk", name=f"xk{li}_{k}")
            nc.vector.tensor_tensor(
                xk, x_sb, ewb[:P, k, None, :].to_broadcast((P, IT, BS)), OP.mult
            )
            return xk

        def open_chunks(li, chunks, bl_sb):
            zps = []
            oc0 = 0
            for ci, ocsz in enumerate(chunks):
                zp = zpsum.tile([BS, 512], F32, tag="z", name=f"zp{li}_{ci}")[:, :ocsz]
                if bl_sb is not None:
                    nc.tensor.matmul(
                        zp, lhsT=ew_sb, rhs=bl_sb[:, oc0 : oc0 + ocsz],
                        start=True, stop=False,
                    )
                zps.append((zp, oc0, ocsz))
                oc0 += ocsz
            return zps

        def close_chunk(li, last, zp, oc0, ocsz, nx_sb):
            if last:
                y_sb = hpool.tile([BS, 256], BF16, tag="y", name="y_sb")[:, :ocsz]
                nc.vector.tensor_copy(out=y_sb, in_=zp)
                nc.sync.dma_start(y_ext[:, oc0 : oc0 + ocsz], y_sb)
            else:
                # h = (max(z,0) - 1) + exp(min(z,0))   (= elu(z))
                m = hscr.tile([BS, 512], F32, tag="hm", name="hm")[:, :ocsz]
                nc.vector.tensor_scalar(m, zp, 0.0, None, OP.min)
                e = hscr.tile([BS, 512], F32, tag="he", name="he")[:, :ocsz]
                nc.scalar.activation(e, m, AF.Exp)
                r = hscr.tile([BS, 512], F32, tag="hr", name="hr")[:, :ocsz]
                nc.vector.tensor_scalar(r, zp, 0.0, -1.0, OP.max, OP.add)
                h = hpool.tile([BS, 512], BF16, tag="hh", name="hh")[:, :ocsz]
                nc.vector.tensor_tensor(h, r, e, OP.add)
                # transpose each 128-col block into next layer's input layout
                for j in range(ocsz // 128):
                    tp = tpsum.tile([128, BS], BF16, tag="tr")
                    nc.tensor.transpose(tp, h[:, j * 128 : (j + 1) * 128], identb)
                    nc.vector.tensor_copy(out=nx_sb[:, (oc0 // 128) + j, :], in_=tp)

        x_sb = x1_sb
        # ---- layers 1 and 2: expert-major (weights stream per expert) ----
        for li, (P, IT, O, chunks) in enumerate((L1, L2)):
            if b_ext[li] is not None:
                bl_sb = gat.tile([K, O], F32, tag=f"bias{li}")
                nc.sync.dma_start(bl_sb, b_ext[li][:])
            else:
                bl_sb = None
            nx_sb = xpool.tile([128, O // 128, BS], BF16, tag=f"x{li + 2}")
            zps = open_chunks(li, chunks, bl_sb)
            for k in range(K):
                if li == 0:
                    if k > 0:
                        w_sb = w1p.tile([120, 4, HID], BF16, tag="w1", name=f"w1_{k}")
                        nc.sync.dma_start(w_sb, w1_ext[k])
                    else:
                        w_sb = w1_tiles[0]
                else:
                    w_sb = w2p.tile([128, 8, HID], BF16, tag="w2", name=f"w2_{k}")
                    nc.sync.dma_start(w_sb, w2_ext[k])
                xk = make_xk(x_sb, P, IT, k, li)
                for zp, occ, ocsz in zps:
                    for it in range(IT):
                        nc.tensor.matmul(
                            zp,
                            lhsT=xk[:, it, :],
                            rhs=w_sb[:, it, occ : occ + ocsz],
                            start=(k == 0 and it == 0 and bl_sb is None),
                            stop=(k == K - 1 and it == IT - 1),
                        )
            for zp, oc0, ocsz in zps:
                close_chunk(li, False, zp, oc0, ocsz, nx_sb)
            x_sb = nx_sb

        # ---- layer 3: chunk-major ----
        P, IT, O, chunks = L3
        if b_ext[2] is not None:
            bl_sb = gat.tile([K, O], F32, tag="bias2")
            nc.sync.dma_start(bl_sb, b_ext[2][:])
        else:
            bl_sb = None
        # DMA issue order = transfer order: all of chunk a, then (b_k, c_k)
        # pairs.  Chunk a is fully resident (8 bufs) so no recycling stalls.
        w3_sb = [[None] * K for _ in chunks]
        for k in range(K):
            w3_sb[0][k] = w3p[0].tile(
                [128, 8 * chunks[0]], BF16, tag="w3a", name=f"w3a_{k}"
            )
            nc.sync.dma_start(w3_sb[0][k], w3_ext[0][k])
        for k in range(K):
            for ci in (1, 2):
                w3_sb[ci][k] = w3p[ci].tile(
                    [128, 8 * chunks[ci]], BF16, tag=f"w3{ci}", name=f"w3{ci}_{k}"
                )
                nc.sync.dma_start(w3_sb[ci][k], w3_ext[ci][k])
        zps = open_chunks(2, chunks, bl_sb)
        xk3 = [None] * K
        for ci, (zp, occ, ocsz) in enumerate(zps):
            for k in range(K):
                if ci == 0:
                    xk3[k] = make_xk(x_sb, P, IT, k, 2)
                csz = chunks[ci]
                w_sb = w3_sb[ci][k]
                for it in range(IT):
                    nc.tensor.matmul(
                        zp,
                        lhsT=xk3[k][:, it, :],
                        rhs=w_sb[:, it * csz : it * csz + ocsz],
                        start=(k == 0 and it == 0 and bl_sb is None),
                        stop=(k == K - 1 and it == IT - 1),
                    )
            close_chunk(2, True, zp, occ, ocsz, None)

    nc.compile()
    return nc


def TileCtx(nc):
    return tile.TileContext(nc)


_PROG_CACHE: dict = {}


def _get_program(with_bias):
    key = tuple(with_bias)
    if key not in _PROG_CACHE:
        _PROG_CACHE[key] = _build_program(key)
    return _PROG_CACHE[key]


def _prep_w(W, P, IT):
    # [K, O, I] -> [K, P, IT, O] with element [k,p,it,o] = W[k,o,it*P+p]
    Kk, O, I = W.shape
    Wt = W.transpose(0, 2, 1).reshape(Kk, IT, P, O).transpose(0, 2, 1, 3)
    return np.ascontiguousarray(Wt.astype(ml_dtypes.bfloat16))


def kernel(
    x_main, x_gate, g1_w, g1_b, g2_w, g2_b, g3_w, g3_b,
    W1, b1, W2, b2, W3, b3,
):
    x_main = np.asarray(x_main, np.float32)
    x_gate = np.asarray(x_gate, np.float32)
    g1_w = np.asarray(g1_w, np.float32)
    g1_b = np.asarray(g1_b, np.float32)
    g2_w = np.asarray(g2_w, np.float32)
    g2_b = np.asarray(g2_b, np.float32)
    g3_w = np.asarray(g3_w, np.float32)
    g3_b = np.asarray(g3_b, np.float32)
    W1 = np.asarray(W1, np.float32)
    b1 = np.asarray(b1, np.float32)
    W2 = np.asarray(W2, np.float32)
    b2 = np.asarray(b2, np.float32)
    W3 = np.asarray(W3, np.float32)
    b3 = np.asarray(b3, np.float32)

    with_bias = (bool(b1.any()), bool(b2.any()), bool(b3.any()))
    nc = _get_program(with_bias)

    # shared (per-core-identical) part of the input blob: groups 5..6
    pg = np.zeros((128, 2, BS), np.float32)
    pg[:, 0, 0:GHID] = g1_w.T
    pg[0:GHID, 0, GHID : 2 * GHID] = g2_w.T
    pg[0:GHID, 1, 0:K] = g3_w.T
    pg[0:GHID, 1, K] = g1_b
    pg[0:GHID, 1, K + 1] = g2_b - g2_w.sum(1)
    pg[0, 1, 16 : 16 + K] = g3_b - g3_w.sum(1)

    w3 = _prep_w(W3, 128, 8)  # [K, 128, 8, 640]
    shared = {
        "w1": _prep_w(W1, 120, 4),
        "w2": _prep_w(W2, 128, 8),
    }
    oc0 = 0
    for c, sz in zip("abc", L3[3]):
        shared[f"w3{c}"] = np.ascontiguousarray(
            w3[:, :, :, oc0 : oc0 + sz].reshape(K, 128, 8 * sz)
        )
        oc0 += sz
    for name, b, flag in (("b1", b1, with_bias[0]), ("b2", b2, with_bias[1]),
                          ("b3", b3, with_bias[2])):
        if flag:
            shared[name] = np.ascontiguousarray(b)

    in_maps = []
    for s in range(NCORES):
        blob = np.zeros((128, IB_GROUPS, BS), np.float32)
        xm_s = x_main[s * BS : (s + 1) * BS].T  # [480, BS]
        blob[0:120, 0:4, :] = xm_s.reshape(4, 120, BS).transpose(1, 0, 2)
        blob[:, 4, :] = x_gate[s * BS : (s + 1) * BS].T
        blob[:, 5:7, :] = pg
        in_maps.append({**shared, "inp": blob})

    global _last_in_maps
    _last_in_maps = in_maps
    res = run_bass_kernel_spmd(nc, in_maps, list(range(NCORES))).results
    return np.concatenate(
        [np.asarray(res[s]["y"]).astype(np.float32) for s in range(NCORES)], axis=0
    )


_last_in_maps = None
